# Initial kernel scaffold
#
"""DistanceLoss kernel for 8 Trainium2 NeuronCores.

Reference computation (T=64, H=32, W=8, B=2048):
    belongs = target.T                              # [T, B] in {0,1}
    iwd  = sum_w inner_window_distances             # [T, H, B]
    cow  = sum_w outer_window_distances             # [T, H, B]
    bl   = belongs*(1-cont)*(ofd + iwd)             # [T, H, B]
    nbl  = (1-belongs)*cont*(ifd + cow)             # [T, H, B]
    loss = mean_b sum_t [ min_h bl + max_h nbl ]

Because c1 = belongs*(1-cont) and c2 = (1-belongs)*cont are constant over h
and take values in {0,1}:  min_h bl == c1 * min_h(ofd+iwd)  and
max_h nbl == c2 * max_h(ifd+cow)  exactly.

Sharding: T is split 8 ways (8 towns per core); per-core slabs of the two
big [T,H,W,B] tensors are contiguous 16.75 MB regions -> maximal DMA
efficiency.  Each core computes a partial [B] loss vector summed over its 8
towns; the host adds the 8 partials and takes the mean.

V3 dataflow per (side, th) slab (rows r=(t4,h32), full b=2048):
  1. two DMAs of [128, (w4 b2048)] (w-split keeps 32 KB contiguous rows)
  2. DVE tree: 7 adds over w + frame add -> a [128, 2048]
     (contiguous tensor_tensor adds measured ~0.8 ns/elem vs 1.7 for the
      strided reduce this replaces)
  3. PE transposes 128x128 blocks, 4 per PSUM bank
  4. DVE reduce min (max for ow side) over h per bank -> m1/m2
  5. tiny final combine with c1/c2 from target/containment, reduce over t
  6. z[p, bc] = partial loss for b = bc*128+p  ->  host mean
"""

import numpy as np

T, H, W, B = 64, 32, 8, 2048
NCORES = 8
TL = T // NCORES          # 8 local towns per core
NBC = B // 128            # 16 batch chunks of 128

_CACHE = {}


def _build_program():
    import concourse.bass as bass
    import concourse.tile as tile
    from concourse import bacc, mybir

    f32 = mybir.dt.float32
    u8 = mybir.dt.uint8
    AX = mybir.AxisListType
    OP = mybir.AluOpType

    nc = bacc.Bacc()
    iw = nc.declare_dram_parameter("iw", [TL, H, W, B], f32, isOutput=False)
    ow = nc.declare_dram_parameter("ow", [TL, H, W, B], f32, isOutput=False)
    ofd = nc.declare_dram_parameter("ofd", [TL, H, B], f32, isOutput=False)
    ifd = nc.declare_dram_parameter("ifd", [TL, H, B], f32, isOutput=False)
    cont = nc.declare_dram_parameter("cont", [TL, B], f32, isOutput=False)
    tgt = nc.declare_dram_parameter("tgt", [B, TL], u8, isOutput=False)
    z = nc.declare_dram_parameter("z", [128, NBC], f32, isOutput=True)

    ident = nc.inline_tensor(np.eye(128, dtype=np.float32), name="ident128")

    with tile.TileContext(nc) as tc:
        with (
            tc.tile_pool(name="const", bufs=1) as const_pool,
            tc.tile_pool(name="big", bufs=6) as big_pool,
            tc.tile_pool(name="frame", bufs=4) as frame_pool,
            tc.tile_pool(name="tmp", bufs=4) as tmp_pool,
            tc.tile_pool(name="atile", bufs=2) as a_pool,
            tc.tile_pool(name="mres", bufs=1) as m_pool,
            tc.tile_pool(name="fin", bufs=1) as fin_pool,
            tc.tile_pool(name="ps", bufs=8, space="PSUM") as psum_pool,
        ):
            identt = const_pool.tile([128, 128], f32)
            nc.sync.dma_start(identt[:], ident[:, :])
            identc = const_pool.tile([128, 128], f32)
            nc.vector.tensor_copy(identc[:], identt[:])

            # m1/m2: col = bc*TL + t
            m1 = m_pool.tile([128, NBC * TL], f32, tag="m1")
            m2 = m_pool.tile([128, NBC * TL], f32, tag="m2")


            BH = B // 2
            frs = {}
            for side in range(2):
                src4 = iw if side == 0 else ow
                src3 = ofd if side == 0 else ifd
                mdst = m1 if side == 0 else m2
                red_op = OP.min if side == 0 else OP.max
                mview = mdst[:].rearrange("p (c t) -> p c t", t=TL)

                for th in range(2):
                    t0 = th * 4
                    for bh in range(2):
                        b0 = bh * BH
                        # two w-split chunks: [128=(t4,h32), (w4 b1024)],
                        # 2 MB, 16 KB contiguous per partition row
                        bts = []
                        for wh in range(2):
                            bt = big_pool.tile([128, 4 * BH], f32, tag="big")
                            nc.sync.dma_start(
                                bt[:].rearrange("p (w b) -> p w b", w=4),
                                src4[
                                    t0 : t0 + 4, :, 4 * wh : 4 * wh + 4,
                                    b0 : b0 + BH,
                                ].rearrange("t h w b -> (t h) w b"),
                            )
                            bts.append(bt)
                        if bh == 0:
                            # frame DMA between the two b-halves: arrives in
                            # time for this half's chain end
                            fr = frame_pool.tile([128, B], f32, tag="fr")
                            nc.sync.dma_start(
                                fr[:],
                                src3[t0 : t0 + 4, :, :].rearrange(
                                    "t h b -> (t h) b"
                                ),
                            )
                            frs[(side, th)] = fr
                        fr = frs[(side, th)]

                        # DVE tree sum over w + frame add
                        t1 = tmp_pool.tile([128, BH], f32, tag="tmp")
                        nc.vector.tensor_add(
                            t1[:], bts[0][:, 0:BH], bts[0][:, BH : 2 * BH]
                        )
                        t2 = tmp_pool.tile([128, BH], f32, tag="tmp")
                        nc.vector.tensor_add(
                            t2[:], bts[0][:, 2 * BH : 3 * BH], bts[0][:, 3 * BH : 4 * BH]
                        )
                        nc.vector.tensor_add(t1[:], t1[:], t2[:])
                        nc.vector.tensor_add(
                            t2[:], bts[1][:, 0:BH], bts[1][:, BH : 2 * BH]
                        )
                        t3 = tmp_pool.tile([128, BH], f32, tag="tmp")
                        nc.vector.tensor_add(
                            t3[:], bts[1][:, 2 * BH : 3 * BH], bts[1][:, 3 * BH : 4 * BH]
                        )
                        nc.vector.tensor_add(t2[:], t2[:], t3[:])
                        nc.vector.tensor_add(t1[:], t1[:], t2[:])
                        a = a_pool.tile([128, BH], f32, tag="a")
                        nc.vector.tensor_add(a[:], t1[:], fr[:, b0 : b0 + BH])

                        # PE transposes: 4 x 128x128 blocks per PSUM bank,
                        # then one batched min/max reduce per bank
                        for g in range(2):
                            pt = psum_pool.tile([128, 512], f32, tag="pt")
                            for q in range(4):
                                lc = g * 4 + q
                                nc.tensor.transpose(
                                    pt[:, q * 128 : (q + 1) * 128],
                                    a[:, lc * 128 : (lc + 1) * 128],
                                    identc[:],
                                )
                            gg = bh * 2 + g
                            nc.vector.tensor_reduce(
                                mview[:, gg * 4 : (gg + 1) * 4, t0 : t0 + 4],
                                pt[:].rearrange(
                                    "p (c t h) -> p c t h", t=4, h=H
                                ),
                                axis=AX.X,
                                op=red_op,
                            )

            # ---- final combine (tgt/cnat DMAs issued last) ----
            tgt8 = fin_pool.tile([128, NBC * TL], u8, tag="tgt8")
            nc.sync.dma_start(
                tgt8[:].rearrange("p (c t) -> p c t", t=TL),
                tgt.rearrange("(c p) t -> p c t", p=128),
            )
            cnat = fin_pool.tile([TL, B], f32, tag="cnat")
            nc.sync.dma_start(cnat[:], cont[:, :])
            bel = fin_pool.tile([128, NBC * TL], f32, tag="bel")
            nc.vector.tensor_copy(bel[:], tgt8[:])

            # containment [t, b] -> [b, (bc t)] via PE transpose (K=8)
            cnatc = fin_pool.tile([TL, B], f32, tag="cnatc")
            nc.vector.tensor_copy(cnatc[:], cnat[:])
            cT = fin_pool.tile([128, NBC * TL], f32, tag="cT")
            cp = psum_pool.tile([128, NBC * TL], f32, tag="pt")
            for bc in range(NBC):
                nc.tensor.transpose(
                    cp[:, bc * TL : (bc + 1) * TL],
                    cnatc[:, bc * 128 : (bc + 1) * 128],
                    identc[0:TL, 0:TL],
                )
            nc.vector.tensor_copy(cT[:], cp[:])

            # c1 = bel*(1-cT) = bel - bel*cT ; c2 = (1-bel)*cT = cT - bel*cT
            bc_t = fin_pool.tile([128, NBC * TL], f32, tag="bct")
            nc.vector.tensor_mul(bc_t[:], bel[:], cT[:])
            c1 = fin_pool.tile([128, NBC * TL], f32, tag="c1")
            nc.vector.tensor_sub(c1[:], bel[:], bc_t[:])
            c2 = fin_pool.tile([128, NBC * TL], f32, tag="c2")
            nc.vector.tensor_sub(c2[:], cT[:], bc_t[:])

            w1 = fin_pool.tile([128, NBC * TL], f32, tag="w1")
            nc.vector.tensor_mul(w1[:], c1[:], m1[:])
            w2 = fin_pool.tile([128, NBC * TL], f32, tag="w2")
            nc.vector.tensor_mul(w2[:], c2[:], m2[:])
            wt = fin_pool.tile([128, NBC * TL], f32, tag="wt")
            nc.vector.tensor_add(wt[:], w1[:], w2[:])

            zb = fin_pool.tile([128, NBC], f32, tag="zb")
            nc.vector.tensor_reduce(
                zb[:],
                wt[:].rearrange("p (c t) -> p c t", t=TL),
                axis=AX.X,
                op=OP.add,
            )
            nc.sync.dma_start(z[:, :], zb[:])

    nc.finalize()
    return nc


def _get_program():
    if "nc" not in _CACHE:
        _CACHE["nc"] = _build_program()
    return _CACHE["nc"]


def kernel(
    inner_window_distances: np.ndarray,
    outer_window_distances: np.ndarray,
    outer_frame_distance: np.ndarray,
    inner_frame_distance: np.ndarray,
    containment: np.ndarray,
    target: np.ndarray,
) -> np.ndarray:
    from concourse.bass_utils import run_bass_kernel_spmd

    nc = _get_program()

    iw = np.ascontiguousarray(inner_window_distances, dtype=np.float32)
    owd = np.ascontiguousarray(outer_window_distances, dtype=np.float32)
    ofd = np.ascontiguousarray(outer_frame_distance, dtype=np.float32)
    ifd = np.ascontiguousarray(inner_frame_distance, dtype=np.float32)
    cont = np.ascontiguousarray(containment, dtype=np.float32)
    tgt = np.ascontiguousarray(target).view(np.uint8)

    core_ids = list(range(NCORES))
    in_maps = []
    for c in core_ids:
        t0, t1 = c * TL, (c + 1) * TL
        in_maps.append(
            {
                "iw": np.ascontiguousarray(iw[t0:t1]),
                "ow": np.ascontiguousarray(owd[t0:t1]),
                "ofd": np.ascontiguousarray(ofd[t0:t1]),
                "ifd": np.ascontiguousarray(ifd[t0:t1]),
                "cont": np.ascontiguousarray(cont[t0:t1]),
                "tgt": np.ascontiguousarray(tgt[:, t0:t1]),
            }
        )

    res = run_bass_kernel_spmd(nc, in_maps, core_ids)

    # z[p, bc] (per core) = partial loss for b = bc*128 + p, summed over the
    # core's 8 towns.  Sum cores, flatten to [B], mean.
    acc = np.zeros((128, NBC), dtype=np.float64)
    for r in res.results:
        acc += r["z"].astype(np.float64)
    loss_b = acc.T.reshape(B)
    return np.float32(loss_b.mean())



# revision 1
# speedup vs baseline: 1.3690x; 1.3690x over previous
"""DistanceLoss kernel for 8 Trainium2 NeuronCores.

Reference computation (T=64, H=32, W=8, B=2048):
    belongs = target.T                              # [T, B] in {0,1}
    iwd  = sum_w inner_window_distances             # [T, H, B]
    cow  = sum_w outer_window_distances             # [T, H, B]
    bl   = belongs*(1-cont)*(ofd + iwd)             # [T, H, B]
    nbl  = (1-belongs)*cont*(ifd + cow)             # [T, H, B]
    loss = mean_b sum_t [ min_h bl + max_h nbl ]

Because c1 = belongs*(1-cont) and c2 = (1-belongs)*cont are constant over h
and take values in {0,1}:  min_h bl == c1 * min_h(ofd+iwd)  and
max_h nbl == c2 * max_h(ifd+cow)  exactly.

Sharding: T is split 8 ways (8 towns per core); per-core slabs of the two
big [T,H,W,B] tensors are contiguous 16.75 MB regions -> maximal DMA
efficiency.  Each core computes a partial [B] loss vector summed over its 8
towns; the host adds the 8 partials and takes the mean.

V3 dataflow per (side, th) slab (rows r=(t4,h32), full b=2048):
  1. two DMAs of [128, (w4 b2048)] (w-split keeps 32 KB contiguous rows)
  2. DVE tree: 7 adds over w + frame add -> a [128, 2048]
     (contiguous tensor_tensor adds measured ~0.8 ns/elem vs 1.7 for the
      strided reduce this replaces)
  3. PE transposes 128x128 blocks, 4 per PSUM bank
  4. DVE reduce min (max for ow side) over h per bank -> m1/m2
  5. tiny final combine with c1/c2 from target/containment, reduce over t
  6. z[p, bc] = partial loss for b = bc*128+p  ->  host mean
"""

import numpy as np

T, H, W, B = 64, 32, 8, 2048
NCORES = 8
TL = T // NCORES          # 8 local towns per core
NBC = B // 128            # 16 batch chunks of 128

_CACHE = {}


def _build_program():
    import concourse.bass as bass
    import concourse.tile as tile
    from concourse import bacc, mybir

    f32 = mybir.dt.float32
    u8 = mybir.dt.uint8
    AX = mybir.AxisListType
    OP = mybir.AluOpType

    nc = bacc.Bacc()
    iw = nc.declare_dram_parameter("iw", [TL, H, W, B], f32, isOutput=False)
    ow = nc.declare_dram_parameter("ow", [TL, H, W, B], f32, isOutput=False)
    ofd = nc.declare_dram_parameter("ofd", [TL, H, B], f32, isOutput=False)
    ifd = nc.declare_dram_parameter("ifd", [TL, H, B], f32, isOutput=False)
    cont = nc.declare_dram_parameter("cont", [TL, B], f32, isOutput=False)
    tgt = nc.declare_dram_parameter("tgt", [B, TL], u8, isOutput=False)
    z = nc.declare_dram_parameter("z", [128, NBC], f32, isOutput=True)

    ident = nc.inline_tensor(np.eye(128, dtype=np.float32), name="ident128")

    with tile.TileContext(nc) as tc:
        with (
            tc.tile_pool(name="const", bufs=1) as const_pool,
            tc.tile_pool(name="big", bufs=6) as big_pool,
            tc.tile_pool(name="frame", bufs=4) as frame_pool,
            tc.tile_pool(name="tmp", bufs=4) as tmp_pool,
            tc.tile_pool(name="atile", bufs=2) as a_pool,
            tc.tile_pool(name="mres", bufs=1) as m_pool,
            tc.tile_pool(name="fin", bufs=1) as fin_pool,
            tc.tile_pool(name="ps", bufs=8, space="PSUM") as psum_pool,
        ):
            identt = const_pool.tile([128, 128], f32)
            nc.sync.dma_start(identt[:], ident[:, :])
            identc = const_pool.tile([128, 128], f32)
            nc.vector.tensor_copy(identc[:], identt[:])

            # m1/m2: col = bc*TL + t
            m1 = m_pool.tile([128, NBC * TL], f32, tag="m1")
            m2 = m_pool.tile([128, NBC * TL], f32, tag="m2")


            BH = B // 2
            frs = {}
            for side in range(2):
                src4 = iw if side == 0 else ow
                src3 = ofd if side == 0 else ifd
                mdst = m1 if side == 0 else m2
                red_op = OP.min if side == 0 else OP.max
                mview = mdst[:].rearrange("p (c t) -> p c t", t=TL)

                for th in range(2):
                    t0 = th * 4
                    for bh in range(2):
                        b0 = bh * BH
                        # two w-split chunks: [128=(t4,h32), (w4 b1024)],
                        # 2 MB, 16 KB contiguous per partition row
                        bts = []
                        for wh in range(2):
                            bt = big_pool.tile([128, 4 * BH], f32, tag="big")
                            nc.sync.dma_start(
                                bt[:].rearrange("p (w b) -> p w b", w=4),
                                src4[
                                    t0 : t0 + 4, :, 4 * wh : 4 * wh + 4,
                                    b0 : b0 + BH,
                                ].rearrange("t h w b -> (t h) w b"),
                            )
                            bts.append(bt)
                        if bh == 0:
                            # frame DMA between the two b-halves: arrives in
                            # time for this half's chain end
                            fr = frame_pool.tile([128, B], f32, tag="fr")
                            nc.sync.dma_start(
                                fr[:],
                                src3[t0 : t0 + 4, :, :].rearrange(
                                    "t h b -> (t h) b"
                                ),
                            )
                            frs[(side, th)] = fr
                        fr = frs[(side, th)]

                        # DVE tree sum over w + frame add
                        t1 = tmp_pool.tile([128, BH], f32, tag="tmp")
                        nc.vector.tensor_add(
                            t1[:], bts[0][:, 0:BH], bts[0][:, BH : 2 * BH]
                        )
                        t2 = tmp_pool.tile([128, BH], f32, tag="tmp")
                        nc.vector.tensor_add(
                            t2[:], bts[0][:, 2 * BH : 3 * BH], bts[0][:, 3 * BH : 4 * BH]
                        )
                        nc.vector.tensor_add(t1[:], t1[:], t2[:])
                        nc.vector.tensor_add(
                            t2[:], bts[1][:, 0:BH], bts[1][:, BH : 2 * BH]
                        )
                        t3 = tmp_pool.tile([128, BH], f32, tag="tmp")
                        nc.vector.tensor_add(
                            t3[:], bts[1][:, 2 * BH : 3 * BH], bts[1][:, 3 * BH : 4 * BH]
                        )
                        nc.vector.tensor_add(t2[:], t2[:], t3[:])
                        nc.vector.tensor_add(t1[:], t1[:], t2[:])
                        a = a_pool.tile([128, BH], f32, tag="a")
                        nc.vector.tensor_add(a[:], t1[:], fr[:, b0 : b0 + BH])

                        # PE transposes: 4 x 128x128 blocks per PSUM bank,
                        # then one batched min/max reduce per bank
                        for g in range(2):
                            pt = psum_pool.tile([128, 512], f32, tag="pt")
                            for q in range(4):
                                lc = g * 4 + q
                                nc.tensor.transpose(
                                    pt[:, q * 128 : (q + 1) * 128],
                                    a[:, lc * 128 : (lc + 1) * 128],
                                    identc[:],
                                )
                            gg = bh * 2 + g
                            nc.vector.tensor_reduce(
                                mview[:, gg * 4 : (gg + 1) * 4, t0 : t0 + 4],
                                pt[:].rearrange(
                                    "p (c t h) -> p c t h", t=4, h=H
                                ),
                                axis=AX.X,
                                op=red_op,
                            )

            # ---- final combine (tgt/cnat DMAs issued last) ----
            tgt8 = fin_pool.tile([128, NBC * TL], u8, tag="tgt8")
            nc.sync.dma_start(
                tgt8[:].rearrange("p (c t) -> p c t", t=TL),
                tgt.rearrange("(c p) t -> p c t", p=128),
            )
            cnat = fin_pool.tile([TL, B], f32, tag="cnat")
            nc.sync.dma_start(cnat[:], cont[:, :])
            bel = fin_pool.tile([128, NBC * TL], f32, tag="bel")
            nc.vector.tensor_copy(bel[:], tgt8[:])

            # containment [t, b] -> [b, (bc t)] via PE transpose (K=8)
            cnatc = fin_pool.tile([TL, B], f32, tag="cnatc")
            nc.vector.tensor_copy(cnatc[:], cnat[:])
            cT = fin_pool.tile([128, NBC * TL], f32, tag="cT")
            cp = psum_pool.tile([128, NBC * TL], f32, tag="pt")
            for bc in range(NBC):
                nc.tensor.transpose(
                    cp[:, bc * TL : (bc + 1) * TL],
                    cnatc[:, bc * 128 : (bc + 1) * 128],
                    identc[0:TL, 0:TL],
                )
            nc.vector.tensor_copy(cT[:], cp[:])

            # c1 = bel*(1-cT) = bel - bel*cT ; c2 = (1-bel)*cT = cT - bel*cT
            bc_t = fin_pool.tile([128, NBC * TL], f32, tag="bct")
            nc.vector.tensor_mul(bc_t[:], bel[:], cT[:])
            c1 = fin_pool.tile([128, NBC * TL], f32, tag="c1")
            nc.vector.tensor_sub(c1[:], bel[:], bc_t[:])
            c2 = fin_pool.tile([128, NBC * TL], f32, tag="c2")
            nc.vector.tensor_sub(c2[:], cT[:], bc_t[:])

            w1 = fin_pool.tile([128, NBC * TL], f32, tag="w1")
            nc.vector.tensor_mul(w1[:], c1[:], m1[:])
            w2 = fin_pool.tile([128, NBC * TL], f32, tag="w2")
            nc.vector.tensor_mul(w2[:], c2[:], m2[:])
            wt = fin_pool.tile([128, NBC * TL], f32, tag="wt")
            nc.vector.tensor_add(wt[:], w1[:], w2[:])

            zb = fin_pool.tile([128, NBC], f32, tag="zb")
            nc.vector.tensor_reduce(
                zb[:],
                wt[:].rearrange("p (c t) -> p c t", t=TL),
                axis=AX.X,
                op=OP.add,
            )
            nc.sync.dma_start(z[:, :], zb[:])

    nc.finalize()
    return nc


def _get_program():
    if "nc" not in _CACHE:
        _CACHE["nc"] = _build_program()
    return _CACHE["nc"]


def kernel(
    inner_window_distances: np.ndarray,
    outer_window_distances: np.ndarray,
    outer_frame_distance: np.ndarray,
    inner_frame_distance: np.ndarray,
    containment: np.ndarray,
    target: np.ndarray,
) -> np.ndarray:
    from concourse.bass_utils import run_bass_kernel_spmd

    nc = _get_program()

    iw = np.ascontiguousarray(inner_window_distances, dtype=np.float32)
    owd = np.ascontiguousarray(outer_window_distances, dtype=np.float32)
    ofd = np.ascontiguousarray(outer_frame_distance, dtype=np.float32)
    ifd = np.ascontiguousarray(inner_frame_distance, dtype=np.float32)
    cont = np.ascontiguousarray(containment, dtype=np.float32)
    tgt = np.ascontiguousarray(target).view(np.uint8)

    core_ids = list(range(NCORES))
    in_maps = []
    for c in core_ids:
        t0, t1 = c * TL, (c + 1) * TL
        in_maps.append(
            {
                "iw": np.ascontiguousarray(iw[t0:t1]),
                "ow": np.ascontiguousarray(owd[t0:t1]),
                "ofd": np.ascontiguousarray(ofd[t0:t1]),
                "ifd": np.ascontiguousarray(ifd[t0:t1]),
                "cont": np.ascontiguousarray(cont[t0:t1]),
                "tgt": np.ascontiguousarray(tgt[:, t0:t1]),
            }
        )

    res = run_bass_kernel_spmd(nc, in_maps, core_ids)

    # z[p, bc] (per core) = partial loss for b = bc*128 + p, summed over the
    # core's 8 towns.  Sum cores, flatten to [B], mean.
    acc = np.zeros((128, NBC), dtype=np.float64)
    for r in res.results:
        acc += r["z"].astype(np.float64)
    loss_b = acc.T.reshape(B)
    return np.float32(loss_b.mean())

